# revision 10
# baseline (speedup 1.0000x reference)
"""MoE routing kernel (CentralSpecificModel) for 8 Trainium2 NeuronCores.

Strategy: expert-parallel with host-side sort-based dispatch.
  - Host: group rows by species, allocate the 8 cores to species
    proportionally to row counts (uniform data -> 2 cores/species),
    pad each core's row set to a multiple of 128, gather + transpose
    its rows to X.T layout, cast activations/weights to fp16.
  - Device (SPMD, one program, per-core data): a dense 2-layer MLP
      Y.T = W2.T @ silu(W1.T @ X.T + b1) + b2
    computed entirely in transposed layout so no on-device transposes
    are needed; fp16 matmuls (1 cycle/row on the PE) accumulate in
    fp32 PSUM; silu+bias on the scalar engine; layer-2 bias on DVE.
  - Host: scatter each core's Y.T back to the original row order.
"""

import sys

sys.path.insert(0, "/opt/trn_rl_repo")

import numpy as np

import concourse.bacc as bacc
import concourse.mybir as mybir
from concourse import tile
from concourse.bass_utils import run_bass_kernel_spmd

N_CORES = 8
NUM_SPECIES = 4
D_IN, D_HID, D_OUT = 256, 1024, 256
KI, KH, KO = D_IN // 128, D_HID // 128, D_OUT // 128  # 2, 8, 2
ROW_TILE = 1024  # rows per pipeline chunk (PSUM-bank-sized halves of 512)
MM_N = 512  # max moving free-dim per matmul (one fp32 PSUM bank)

F32 = mybir.dt.float32
F16 = mybir.dt.float16
AF = mybir.ActivationFunctionType

_program_cache: dict[int, object] = {}


def _build_program(m_rows: int, repeats: int = 1):
    """One-core program for m_rows (multiple of 128) rows of one species.

    repeats > 1 re-emits the whole chunk pipeline that many times (weights
    loaded once) — used by the test harness to measure steady-state HW time
    by slope, cancelling the per-dispatch launch overhead.
    """
    nc = bacc.Bacc(
        "TRN2", target_bir_lowering=False, debug=False, num_devices=N_CORES
    )
    xt = nc.dram_tensor("xt", [D_IN, m_rows], F16, kind="ExternalInput")
    w1 = nc.dram_tensor("w1", [D_IN, D_HID], F16, kind="ExternalInput")
    w2 = nc.dram_tensor("w2", [D_HID, D_OUT], F16, kind="ExternalInput")
    b1a = nc.dram_tensor("b1a", [128, KH], F32, kind="ExternalInput")
    b2a = nc.dram_tensor("b2a", [128, KO], F32, kind="ExternalInput")
    yt = nc.dram_tensor("yt", [D_OUT, m_rows], F32, kind="ExternalOutput")

    # Balanced chunk sizes (multiples of 128, <= ROW_TILE): avoids a
    # degenerate tiny tail chunk whose short matmuls run at the per-MM
    # dispatch floor.
    nch = -(-m_rows // ROW_TILE)
    base = (m_rows // nch) // 128 * 128
    sizes = [base] * nch
    rem = m_rows - base * nch
    i = 0
    while rem:
        sizes[i] += 128
        rem -= 128
        i += 1
    chunks = []
    off = 0
    for cm in sizes:
        chunks.append((off, cm))
        off += cm

    with tile.TileContext(nc) as tc:
        with (
            tc.tile_pool(name="wp", bufs=1) as wp,
            tc.tile_pool(name="xp", bufs=3) as xp,
            tc.tile_pool(name="hp", bufs=3) as hp,
            tc.tile_pool(name="yp", bufs=3) as yp,
            tc.tile_pool(name="ps1", bufs=2, space="PSUM") as ps1,
            tc.tile_pool(name="ps2", bufs=2, space="PSUM") as ps2,
        ):
            w1_sb = wp.tile([128, KI * D_HID], F16, tag="w1")
            nc.sync.dma_start(
                out=w1_sb.rearrange("p (k h) -> p k h", k=KI),
                in_=w1.rearrange("(k p) h -> p k h", p=128),
            )
            w2_sb = wp.tile([128, KH * D_OUT], F16, tag="w2")
            nc.sync.dma_start(
                out=w2_sb.rearrange("p (k o) -> p k o", k=KH),
                in_=w2.rearrange("(k p) o -> p k o", p=128),
            )
            b1_sb = wp.tile([128, KH], F32, tag="b1")
            nc.sync.dma_start(out=b1_sb[:], in_=b1a[:])
            b2_sb = wp.tile([128, KO], F32, tag="b2")
            nc.sync.dma_start(out=b2_sb[:], in_=b2a[:])

            def emit_l2(off, cm, h_sb):
                y_sb = yp.tile([128, KO * cm], F32, tag="y")
                for o in range(KO):
                    ps = ps2.tile([128, cm], F32, tag="ps2")
                    for kk in range(KH):
                        for c0 in range(0, cm, MM_N):
                            w_ = min(MM_N, cm - c0)
                            nc.tensor.matmul(
                                ps[:, c0 : c0 + w_],
                                w2_sb[
                                    :,
                                    kk * D_OUT + o * 128 : kk * D_OUT + o * 128 + 128,
                                ],
                                h_sb[:, kk * cm + c0 : kk * cm + c0 + w_],
                                start=(kk == 0),
                                stop=(kk == KH - 1),
                            )
                    nc.vector.tensor_scalar_add(
                        y_sb[:, o * cm : (o + 1) * cm], ps[:], b2_sb[:, o : o + 1]
                    )
                nc.sync.dma_start(
                    out=yt[:, off : off + cm].rearrange("(k p) m -> p k m", p=128),
                    in_=y_sb.rearrange("p (k m) -> p k m", k=KO),
                )

            # Software pipeline: layer 2 of chunk c-1 is emitted after layer 1
            # of chunk c, so the PE never waits on the silu chain of the chunk
            # it just filled.
            pending = None
            for off, cm in chunks * repeats:
                x_sb = xp.tile([128, KI * cm], F16, tag="x")
                nc.sync.dma_start(
                    out=x_sb.rearrange("p (k m) -> p k m", k=KI),
                    in_=xt[:, off : off + cm].rearrange("(k p) m -> p k m", p=128),
                )
                h_sb = hp.tile([128, KH * cm], F16, tag="h")
                for h in range(KH):
                    ps = ps1.tile([128, cm], F32, tag="ps1")
                    for kk in range(KI):
                        for c0 in range(0, cm, MM_N):
                            w_ = min(MM_N, cm - c0)
                            nc.tensor.matmul(
                                ps[:, c0 : c0 + w_],
                                w1_sb[
                                    :,
                                    kk * D_HID + h * 128 : kk * D_HID + h * 128 + 128,
                                ],
                                x_sb[:, kk * cm + c0 : kk * cm + c0 + w_],
                                start=(kk == 0),
                                stop=(kk == KI - 1),
                            )
                    nc.scalar.activation(
                        h_sb[:, h * cm : (h + 1) * cm],
                        ps[:],
                        AF.Silu,
                        bias=b1_sb[:, h : h + 1],
                    )
                if pending is not None:
                    emit_l2(*pending)
                pending = (off, cm, h_sb)
            emit_l2(*pending)

    nc.compile()
    return nc


def _get_program(m_rows: int, repeats: int = 1):
    key = (m_rows, repeats)
    if key not in _program_cache:
        _program_cache[key] = _build_program(m_rows, repeats)
    return _program_cache[key]


def _allocate_cores(counts: np.ndarray) -> list[int]:
    """Distribute N_CORES cores over species proportionally to row counts;
    every species with rows gets at least one core."""
    active = [s for s in range(NUM_SPECIES) if counts[s] > 0]
    alloc = [0] * NUM_SPECIES
    for s in active:
        alloc[s] = 1
    extra = N_CORES - len(active)
    if extra > 0:
        total = float(sum(counts[s] for s in active))
        shares = [extra * counts[s] / total for s in active]
        floors = [int(x) for x in shares]
        for s, f in zip(active, floors):
            alloc[s] += f
        rem = extra - sum(floors)
        order = sorted(active, key=lambda s: shares[active.index(s)] - floors[active.index(s)], reverse=True)
        for s in order[:rem]:
            alloc[s] += 1
    return alloc


def _shard(x, central_species):
    """Returns per-core (species, row-index array) assignments and M."""
    species = np.asarray(central_species).astype(np.int64).ravel()
    counts = np.bincount(species, minlength=NUM_SPECIES)
    alloc = _allocate_cores(counts)
    order = np.argsort(species, kind="stable")
    assignments = []
    pos = 0
    for s in range(NUM_SPECIES):
        idx_s = order[pos : pos + counts[s]]
        pos += counts[s]
        if alloc[s] == 0:
            continue
        for part in np.array_split(idx_s, alloc[s]):
            assignments.append((s, part))
    assert len(assignments) == N_CORES
    longest = max(len(p) for _, p in assignments)
    m_rows = max(128, -(-longest // 128) * 128)
    return assignments, m_rows


def _prepare(x, central_species, W1, b1, W2, b2):
    x = np.asarray(x, dtype=np.float32)
    W1 = np.asarray(W1, dtype=np.float32)
    b1 = np.asarray(b1, dtype=np.float32)
    W2 = np.asarray(W2, dtype=np.float32)
    b2 = np.asarray(b2, dtype=np.float32)

    assignments, m_rows = _shard(x, central_species)

    w1_f16 = [np.ascontiguousarray(W1[s]).astype(np.float16) for s in range(NUM_SPECIES)]
    w2_f16 = [np.ascontiguousarray(W2[s]).astype(np.float16) for s in range(NUM_SPECIES)]
    b1_arr = [np.ascontiguousarray(b1[s].reshape(KH, 128).T).astype(np.float32) for s in range(NUM_SPECIES)]
    b2_arr = [np.ascontiguousarray(b2[s].reshape(KO, 128).T).astype(np.float32) for s in range(NUM_SPECIES)]

    x16 = x.astype(np.float16)
    in_maps = []
    for s, idx in assignments:
        pad_src = idx[0] if len(idx) else 0
        idx_p = np.concatenate([idx, np.full(m_rows - len(idx), pad_src, dtype=idx.dtype)])
        xt = np.ascontiguousarray(x16[idx_p].T)
        in_maps.append(
            {"xt": xt, "w1": w1_f16[s], "w2": w2_f16[s], "b1a": b1_arr[s], "b2a": b2_arr[s]}
        )
    return assignments, m_rows, in_maps


_runner_cache: dict[int, object] = {}


def _get_runner(m_rows: int):
    """Cached jitted PJRT runner for the m_rows program: avoids re-tracing
    and re-compiling the XLA wrapper on every kernel() call (the NEFF itself
    is also cached). Returns run(in_maps) -> list of per-core {name: np}."""
    if m_rows in _runner_cache:
        return _runner_cache[m_rows]

    import jax
    import jax.numpy as jnp
    from jax.experimental.shard_map import shard_map
    from jax.sharding import Mesh, NamedSharding, PartitionSpec

    from concourse import bass2jax

    nc = _get_program(m_rows)
    bass2jax.install_neuronx_cc_hook()

    partition_name = nc.partition_id_tensor.name if nc.partition_id_tensor else None
    in_names, out_names, out_avals, zero_shapes = [], [], [], []
    for alloc in nc.m.functions[0].allocations:
        if not isinstance(alloc, mybir.MemoryLocationSet):
            continue
        name = alloc.memorylocations[0].name
        if alloc.kind == "ExternalInput":
            if name == partition_name:
                continue
            in_names.append(name)
        elif alloc.kind == "ExternalOutput":
            out_names.append(name)
            shape = tuple(alloc.tensor_shape)
            dtype = mybir.dt.np(alloc.dtype)
            out_avals.append(jax.core.ShapedArray(shape, dtype))
            zero_shapes.append((shape, dtype))
    n_params = len(in_names)
    n_outs = len(out_names)
    all_in_names = tuple(
        in_names + out_names + ([partition_name] if partition_name else [])
    )

    devices = jax.devices()[:N_CORES]
    mesh = Mesh(np.asarray(devices), ("core",))
    spec = PartitionSpec("core")
    shard = NamedSharding(mesh, spec)

    def _body(*args):
        operands = list(args)
        if partition_name is not None:
            operands.append(bass2jax.partition_id_tensor())
        outs = bass2jax._bass_exec_p.bind(
            *operands,
            out_avals=tuple(out_avals),
            in_names=all_in_names,
            out_names=tuple(out_names),
            lowering_input_output_aliases=(),
            sim_require_finite=True,
            sim_require_nnan=True,
            nc=nc,
        )
        return tuple(outs)

    donate = tuple(range(n_params, n_params + n_outs))
    sharded = jax.jit(
        shard_map(
            _body,
            mesh=mesh,
            in_specs=(spec,) * (n_params + n_outs),
            out_specs=(spec,) * n_outs,
            check_rep=False,
        ),
        donate_argnums=donate,
        keep_unused=True,
    )
    zeros_makers = [
        jax.jit(
            lambda shape=shape, dtype=dtype: jnp.zeros(
                (N_CORES * shape[0],) + tuple(shape[1:]), dtype
            ),
            out_shardings=shard,
        )
        for shape, dtype in zero_shapes
    ]

    def run(in_maps):
        concat_in = [
            jax.device_put(
                np.concatenate(
                    [np.asarray(in_maps[c][nm]) for c in range(N_CORES)], axis=0
                ),
                shard,
            )
            for nm in in_names
        ]
        outs = sharded(*concat_in, *[zm() for zm in zeros_makers])
        return [
            {
                nm: np.asarray(outs[i]).reshape(N_CORES, *out_avals[i].shape)[c]
                for i, nm in enumerate(out_names)
            }
            for c in range(N_CORES)
        ]

    _runner_cache[m_rows] = run
    return run


def kernel(x, central_species, W1, b1, W2, b2):
    n_rows = np.asarray(x).shape[0]
    assignments, m_rows, in_maps = _prepare(x, central_species, W1, b1, W2, b2)

    try:
        results = _get_runner(m_rows)(in_maps)
    except Exception:
        # Fallback: the generic SPMD execution path.
        nc = _get_program(m_rows)
        results = run_bass_kernel_spmd(nc, in_maps, list(range(N_CORES))).results

    out = np.zeros((n_rows, D_OUT), dtype=np.float32)
    for (s, idx), r in zip(assignments, results):
        out[idx] = r["yt"].T[: len(idx)]
    return out


# revision 15
# speedup vs baseline: 3.2800x; 3.2800x over previous
"""MoE routing kernel (CentralSpecificModel) for 8 Trainium2 NeuronCores.

Strategy: expert-parallel with host-side sort-based dispatch.
  - Host: group rows by species, allocate the 8 cores to species
    proportionally to row counts (uniform data -> 2 cores/species),
    pad each core's row set to a multiple of 128, gather + transpose
    its rows to X.T layout, cast activations/weights to fp16.
  - Device (SPMD, one program, per-core data): a dense 2-layer MLP
      Y.T = W2.T @ silu(W1.T @ X.T + b1) + b2
    computed entirely in transposed layout so no on-device transposes
    are needed; fp16 matmuls (1 cycle/row on the PE) accumulate in
    fp32 PSUM; silu+bias on the scalar engine; layer-2 bias on DVE.
  - Host: scatter each core's Y.T back to the original row order.
"""

import sys

sys.path.insert(0, "/opt/trn_rl_repo")

import numpy as np

import concourse.bacc as bacc
import concourse.mybir as mybir
from concourse import bass_utils as _bass_utils
from concourse import tile
from concourse.bass_utils import run_bass_kernel_spmd

import os as _os

if _os.environ.get("KERNEL_LDW_OPT", "0") == "1" and not getattr(
    _bass_utils, "_ldw_opt_patched", False
):
    # Walrus's LDWEIGHTS scheduling optimization is off by default in this
    # compile path; without it every matmul serializes its weight load
    # (~25% PE throughput loss on N=512 fp16 streams).
    _orig_run_command = _bass_utils.run_command

    def _run_command_ldw(cmd, *a, **kw):
        if isinstance(cmd, list):
            cmd = [
                "--enable-ldw-opt=true" if c == "--enable-ldw-opt=false" else c
                for c in cmd
            ]
        return _orig_run_command(cmd, *a, **kw)

    _bass_utils.run_command = _run_command_ldw
    _bass_utils._ldw_opt_patched = True

N_CORES = 8
NUM_SPECIES = 4
D_IN, D_HID, D_OUT = 256, 1024, 256
KI, KH, KO = D_IN // 128, D_HID // 128, D_OUT // 128  # 2, 8, 2
ROW_TILE = 1024  # rows per pipeline chunk (PSUM-bank-sized halves of 512)
MM_N = 512  # max moving free-dim per matmul (one fp32 PSUM bank)

F32 = mybir.dt.float32
F16 = mybir.dt.float16
AF = mybir.ActivationFunctionType

_program_cache: dict[int, object] = {}


def _build_program(m_rows: int, repeats: int = 1):
    """One-core program for m_rows (multiple of 128) rows of one species.

    repeats > 1 re-emits the whole chunk pipeline that many times (weights
    loaded once) — used by the test harness to measure steady-state HW time
    by slope, cancelling the per-dispatch launch overhead.
    """
    nc = bacc.Bacc(
        "TRN2", target_bir_lowering=False, debug=False, num_devices=N_CORES
    )
    xt = nc.dram_tensor("xt", [D_IN, m_rows], F16, kind="ExternalInput")
    w1 = nc.dram_tensor("w1", [D_IN, D_HID], F16, kind="ExternalInput")
    w2 = nc.dram_tensor("w2", [D_HID, D_OUT], F16, kind="ExternalInput")
    b1a = nc.dram_tensor("b1a", [128, KH], F32, kind="ExternalInput")
    b2a = nc.dram_tensor("b2a", [128, KO], F32, kind="ExternalInput")
    yt = nc.dram_tensor("yt", [D_OUT, m_rows], F32, kind="ExternalOutput")

    # Balanced chunk sizes (multiples of 128, <= ROW_TILE): avoids a
    # degenerate tiny tail chunk whose short matmuls run at the per-MM
    # dispatch floor.
    nch = -(-m_rows // ROW_TILE)
    base = (m_rows // nch) // 128 * 128
    sizes = [base] * nch
    rem = m_rows - base * nch
    i = 0
    while rem:
        sizes[i] += 128
        rem -= 128
        i += 1
    chunks = []
    off = 0
    for cm in sizes:
        chunks.append((off, cm))
        off += cm

    with tile.TileContext(nc) as tc:
        with (
            tc.tile_pool(name="wp", bufs=1) as wp,
            tc.tile_pool(name="xp", bufs=3) as xp,
            tc.tile_pool(name="hp", bufs=3) as hp,
            tc.tile_pool(name="yp", bufs=3) as yp,
            tc.tile_pool(name="ps1", bufs=3, space="PSUM") as ps1,
            tc.tile_pool(name="ps2", bufs=2, space="PSUM") as ps2,
        ):
            w1_sb = wp.tile([128, KI * D_HID], F16, tag="w1")
            nc.sync.dma_start(
                out=w1_sb.rearrange("p (k h) -> p k h", k=KI),
                in_=w1.rearrange("(k p) h -> p k h", p=128),
            )
            w2_sb = wp.tile([128, KH * D_OUT], F16, tag="w2")
            nc.sync.dma_start(
                out=w2_sb.rearrange("p (k o) -> p k o", k=KH),
                in_=w2.rearrange("(k p) o -> p k o", p=128),
            )
            b1_sb = wp.tile([128, KH], F32, tag="b1")
            nc.sync.dma_start(out=b1_sb[:], in_=b1a[:])
            b2_sb = wp.tile([128, KO], F32, tag="b2")
            nc.sync.dma_start(out=b2_sb[:], in_=b2a[:])

            def emit_l2(off, cm, h_sb):
                y_sb = yp.tile([128, KO * cm], F32, tag="y")
                for o in range(KO):
                    for c0 in range(0, cm, MM_N):
                        w_ = min(MM_N, cm - c0)
                        ps = ps2.tile([128, MM_N], F32, tag="ps2")
                        for kk in range(KH):
                            nc.tensor.matmul(
                                ps[:, :w_],
                                w2_sb[
                                    :,
                                    kk * D_OUT + o * 128 : kk * D_OUT + o * 128 + 128,
                                ],
                                h_sb[:, kk * cm + c0 : kk * cm + c0 + w_],
                                start=(kk == 0),
                                stop=(kk == KH - 1),
                            )
                        nc.vector.tensor_scalar_add(
                            y_sb[:, o * cm + c0 : o * cm + c0 + w_],
                            ps[:, :w_],
                            b2_sb[:, o : o + 1],
                        )
                nc.sync.dma_start(
                    out=yt[:, off : off + cm].rearrange("(k p) m -> p k m", p=128),
                    in_=y_sb.rearrange("p (k m) -> p k m", k=KO),
                )

            # Software pipeline: layer 2 of chunk c-1 is emitted after layer 1
            # of chunk c, so the PE never waits on the silu chain of the chunk
            # it just filled.
            pending = None
            for off, cm in chunks * repeats:
                x_sb = xp.tile([128, KI * cm], F16, tag="x")
                nc.sync.dma_start(
                    out=x_sb.rearrange("p (k m) -> p k m", k=KI),
                    in_=xt[:, off : off + cm].rearrange("(k p) m -> p k m", p=128),
                )
                h_sb = hp.tile([128, KH * cm], F16, tag="h")
                for h in range(KH):
                    ps = ps1.tile([128, cm], F32, tag="ps1")
                    for kk in range(KI):
                        for c0 in range(0, cm, MM_N):
                            w_ = min(MM_N, cm - c0)
                            nc.tensor.matmul(
                                ps[:, c0 : c0 + w_],
                                w1_sb[
                                    :,
                                    kk * D_HID + h * 128 : kk * D_HID + h * 128 + 128,
                                ],
                                x_sb[:, kk * cm + c0 : kk * cm + c0 + w_],
                                start=(kk == 0),
                                stop=(kk == KI - 1),
                            )
                    nc.scalar.activation(
                        h_sb[:, h * cm : (h + 1) * cm],
                        ps[:],
                        AF.Silu,
                        bias=b1_sb[:, h : h + 1],
                    )
                if pending is not None:
                    emit_l2(*pending)
                pending = (off, cm, h_sb)
            emit_l2(*pending)

    nc.compile()
    return nc


def _get_program(m_rows: int, repeats: int = 1):
    key = (m_rows, repeats)
    if key not in _program_cache:
        _program_cache[key] = _build_program(m_rows, repeats)
    return _program_cache[key]


def _allocate_cores(counts: np.ndarray) -> list[int]:
    """Distribute N_CORES cores over species proportionally to row counts;
    every species with rows gets at least one core."""
    active = [s for s in range(NUM_SPECIES) if counts[s] > 0]
    alloc = [0] * NUM_SPECIES
    for s in active:
        alloc[s] = 1
    extra = N_CORES - len(active)
    if extra > 0:
        total = float(sum(counts[s] for s in active))
        shares = [extra * counts[s] / total for s in active]
        floors = [int(x) for x in shares]
        for s, f in zip(active, floors):
            alloc[s] += f
        rem = extra - sum(floors)
        order = sorted(active, key=lambda s: shares[active.index(s)] - floors[active.index(s)], reverse=True)
        for s in order[:rem]:
            alloc[s] += 1
    return alloc


def _shard(x, central_species):
    """Returns per-core (species, row-index array) assignments and M."""
    species = np.asarray(central_species).astype(np.int64).ravel()
    counts = np.bincount(species, minlength=NUM_SPECIES)
    alloc = _allocate_cores(counts)
    order = np.argsort(species, kind="stable")
    assignments = []
    pos = 0
    for s in range(NUM_SPECIES):
        idx_s = order[pos : pos + counts[s]]
        pos += counts[s]
        if alloc[s] == 0:
            continue
        for part in np.array_split(idx_s, alloc[s]):
            assignments.append((s, part))
    assert len(assignments) == N_CORES
    longest = max(len(p) for _, p in assignments)
    m_rows = max(128, -(-longest // 128) * 128)
    return assignments, m_rows


def _prepare(x, central_species, W1, b1, W2, b2):
    x = np.asarray(x, dtype=np.float32)
    W1 = np.asarray(W1, dtype=np.float32)
    b1 = np.asarray(b1, dtype=np.float32)
    W2 = np.asarray(W2, dtype=np.float32)
    b2 = np.asarray(b2, dtype=np.float32)

    assignments, m_rows = _shard(x, central_species)

    w1_f16 = [np.ascontiguousarray(W1[s]).astype(np.float16) for s in range(NUM_SPECIES)]
    w2_f16 = [np.ascontiguousarray(W2[s]).astype(np.float16) for s in range(NUM_SPECIES)]
    b1_arr = [np.ascontiguousarray(b1[s].reshape(KH, 128).T).astype(np.float32) for s in range(NUM_SPECIES)]
    b2_arr = [np.ascontiguousarray(b2[s].reshape(KO, 128).T).astype(np.float32) for s in range(NUM_SPECIES)]

    x16 = x.astype(np.float16)
    in_maps = []
    for s, idx in assignments:
        pad_src = idx[0] if len(idx) else 0
        idx_p = np.concatenate([idx, np.full(m_rows - len(idx), pad_src, dtype=idx.dtype)])
        xt = np.ascontiguousarray(x16[idx_p].T)
        in_maps.append(
            {"xt": xt, "w1": w1_f16[s], "w2": w2_f16[s], "b1a": b1_arr[s], "b2a": b2_arr[s]}
        )
    return assignments, m_rows, in_maps


_runner_cache: dict[int, object] = {}


def _get_runner(m_rows: int):
    """Cached jitted PJRT runner for the m_rows program: avoids re-tracing
    and re-compiling the XLA wrapper on every kernel() call (the NEFF itself
    is also cached). Returns run(in_maps) -> list of per-core {name: np}."""
    if m_rows in _runner_cache:
        return _runner_cache[m_rows]

    import jax
    import jax.numpy as jnp
    from jax.experimental.shard_map import shard_map
    from jax.sharding import Mesh, NamedSharding, PartitionSpec

    from concourse import bass2jax

    nc = _get_program(m_rows)
    bass2jax.install_neuronx_cc_hook()

    partition_name = nc.partition_id_tensor.name if nc.partition_id_tensor else None
    in_names, out_names, out_avals, zero_shapes = [], [], [], []
    for alloc in nc.m.functions[0].allocations:
        if not isinstance(alloc, mybir.MemoryLocationSet):
            continue
        name = alloc.memorylocations[0].name
        if alloc.kind == "ExternalInput":
            if name == partition_name:
                continue
            in_names.append(name)
        elif alloc.kind == "ExternalOutput":
            out_names.append(name)
            shape = tuple(alloc.tensor_shape)
            dtype = mybir.dt.np(alloc.dtype)
            out_avals.append(jax.core.ShapedArray(shape, dtype))
            zero_shapes.append((shape, dtype))
    n_params = len(in_names)
    n_outs = len(out_names)
    all_in_names = tuple(
        in_names + out_names + ([partition_name] if partition_name else [])
    )

    devices = jax.devices()[:N_CORES]
    mesh = Mesh(np.asarray(devices), ("core",))
    spec = PartitionSpec("core")
    shard = NamedSharding(mesh, spec)

    def _body(*args):
        operands = list(args)
        if partition_name is not None:
            operands.append(bass2jax.partition_id_tensor())
        outs = bass2jax._bass_exec_p.bind(
            *operands,
            out_avals=tuple(out_avals),
            in_names=all_in_names,
            out_names=tuple(out_names),
            lowering_input_output_aliases=(),
            sim_require_finite=True,
            sim_require_nnan=True,
            nc=nc,
        )
        return tuple(outs)

    donate = tuple(range(n_params, n_params + n_outs))
    sharded = jax.jit(
        shard_map(
            _body,
            mesh=mesh,
            in_specs=(spec,) * (n_params + n_outs),
            out_specs=(spec,) * n_outs,
            check_rep=False,
        ),
        donate_argnums=donate,
        keep_unused=True,
    )
    zeros_makers = [
        jax.jit(
            lambda shape=shape, dtype=dtype: jnp.zeros(
                (N_CORES * shape[0],) + tuple(shape[1:]), dtype
            ),
            out_shardings=shard,
        )
        for shape, dtype in zero_shapes
    ]

    def run(in_maps):
        concat_in = [
            jax.device_put(
                np.concatenate(
                    [np.asarray(in_maps[c][nm]) for c in range(N_CORES)], axis=0
                ),
                shard,
            )
            for nm in in_names
        ]
        outs = sharded(*concat_in, *[zm() for zm in zeros_makers])
        return [
            {
                nm: np.asarray(outs[i]).reshape(N_CORES, *out_avals[i].shape)[c]
                for i, nm in enumerate(out_names)
            }
            for c in range(N_CORES)
        ]

    _runner_cache[m_rows] = run
    return run


def kernel(x, central_species, W1, b1, W2, b2):
    n_rows = np.asarray(x).shape[0]
    assignments, m_rows, in_maps = _prepare(x, central_species, W1, b1, W2, b2)

    try:
        results = _get_runner(m_rows)(in_maps)
    except Exception:
        # Fallback: the generic SPMD execution path.
        nc = _get_program(m_rows)
        results = run_bass_kernel_spmd(nc, in_maps, list(range(N_CORES))).results

    out = np.zeros((n_rows, D_OUT), dtype=np.float32)
    for (s, idx), r in zip(assignments, results):
        out[idx] = r["yt"].T[: len(idx)]
    return out
